# revision 28
# baseline (speedup 1.0000x reference)
"""Multi-head causal attention (B=8, T=1024, C=1024, H=16, hs=64) on 8 trn2 cores.

Data-parallel over batch: core b computes full attention for x[b].

Device algorithm (per core), matmuls bf16 inputs / fp32 PSUM accum:
  - xT [C, T] resident in SBUF (host pre-transposed, bf16); ~24 dummy
    matmuls at t=0 warm the PE HAM clock gate during the input DMAs.
  - software-pipelined pair loop so ScalarE exp work (~90us, the
    secondary critical engine) starts at ~12us and overlaps the PE
    stream; AV(p-1) interleaves between scores(p) and proj(p+1) so each
    pair's normalize chain hides under the next projection; the tail
    keeps AV(5..7) (~11us of PE work) after scores(7) to cover exp(7).
  - scores computed transposed (scT[s, t]) per head-pair in two t-passes
    of 512 cols: PSUM tile [128, 2(head), 512] per (s-tile, pass); the two
    heads' K=64 matmuls run concurrently in PE row groups (0,0)/(64,0)
    into different PSUM banks; ONE exp per tile on ScalarE (free AP
    [2, width], scale=1/8) -> mega es tiles [128, s-tile, head, 512].
  - causal diagonal 128x128 blocks masked by one tril multiply per
    (s-tile): [128, 2, 128] DVE op against a replicated tril.
  - out^T[65, t] accumulated over s chunks: lhsT = [v | 1], rhs = expT.
    Row 64 = sum(exp) = softmax denominator.
  - normalize per (pair, head): copy both 512-halves into avs [65, 1024],
    DVE reciprocal of the denom row, GpSimd partition-broadcast, DVE
    multiply -> osb bf16, single 128KB DMA to out[h].
"""

import numpy as np
import ml_dtypes

import concourse.bass as bass
import concourse.mybir as mybir
from concourse import bacc
from concourse.tile import TileContext
from concourse.bass import ds, ts
from concourse.bass_utils import run_bass_kernel_spmd
from concourse.masks import make_upper_triangular

BF16 = mybir.dt.bfloat16
F32 = mybir.dt.float32

B, T, C, H, HS = 8, 1024, 1024, 16, 64
P = 128
CK = C // P       # 8 contraction chunks
TT = T // P       # 8 s tiles
PAIRS = H // 2    # 8 head pairs
HALF = 512

_BUILT = None


def build_nc():
    nc = bacc.Bacc("TRN2", target_bir_lowering=False, debug=False)
    # [p, c, t] : xT[C, T] chunked; partition p, chunk c -> row 128c+p of xT
    xt = nc.dram_tensor("xt", [P, CK, T], BF16, kind="ExternalInput")
    # [proj(q,k), pair, p, c, f] : lhsT chunks, f = 2 heads x 64 stacked
    wqk = nc.dram_tensor("wqk", [2, PAIRS, P, CK, P], BF16, kind="ExternalInput")
    # [p, c, pair, f]
    wv = nc.dram_tensor("wv", [P, CK, PAIRS, P], BF16, kind="ExternalInput")
    # out^T per head: [head, d, t]; host transposes to [T, H*HS]
    out = nc.dram_tensor("out", [H, HS, T], BF16, kind="ExternalOutput")

    with TileContext(nc) as tc:
        with (
            tc.tile_pool(name="const", bufs=1) as constp,
            tc.tile_pool(name="wpool", bufs=6) as wpool,
            tc.tile_pool(name="qkpool", bufs=6) as qkp,
            tc.tile_pool(name="espool", bufs=4) as espA,
            tc.tile_pool(name="espoolB", bufs=4) as espB,
            tc.tile_pool(name="normpool", bufs=2) as normp,
            tc.tile_pool(name="psA", bufs=2, space="PSUM") as psA,
            tc.tile_pool(name="psSc", bufs=2, space="PSUM") as psSc,
            tc.tile_pool(name="psV", bufs=2, space="PSUM") as psV,
        ):
            # ---- PE warm-up + ScalarE table preload (run during DMA-in) ----
            maskrep = constp.tile([P, 2, P], BF16)
            for j in range(2):
                make_upper_triangular(nc, maskrep[:, j, :], val=1.0, diag=True)
            # ~24 trivial matmuls keep the PE busy >3.4us so the HAM clock
            # gate opens before the real stream starts (else the first
            # ~20us of projections run at 1.2GHz).
            dummy = psV.tile([HS + 1, HALF], F32, tag="av", name="warmup")
            for r in range(24):
                nc.tensor.matmul(
                    dummy[0:1, 0:2 * P],
                    maskrep[:, 0, 0:1],
                    maskrep[:, :, :],
                )
            # warm the ScalarE exp table during the input-DMA window
            scratch = constp.tile([1, 8], BF16)
            nc.gpsimd.memset(scratch[:, :], 0.0)
            nc.scalar.activation(scratch[:, :], scratch[:, :],
                                 mybir.ActivationFunctionType.Exp)

            def dma_w(pair):
                wq_sb = wpool.tile([P, CK, P], BF16, tag="w", name=f"wq{pair}")
                nc.sync.dma_start(wq_sb[:, :, :], wqk[0, pair, :, :, :])
                wk_sb = wpool.tile([P, CK, P], BF16, tag="w", name=f"wk{pair}")
                nc.sync.dma_start(wk_sb[:, :, :], wqk[1, pair, :, :, :])
                return wq_sb, wk_sb

            # pair-0 weights first (small), then xt chunk-by-chunk in
            # separate tiles so proj(0)'s c=0 matmul starts on chunk 0.
            w_sb = {0: dma_w(0)}
            xt_sb = [constp.tile([P, T], BF16, name=f"xt{c}") for c in range(CK)]
            for c in range(CK):
                nc.sync.dma_start(xt_sb[c][:, :], xt[:, c, :])
            w_sb[1] = dma_w(1)
            w_sb[2] = dma_w(2)
            w_sb[3] = dma_w(3)

            # pair-group-major: [p, c, pg, 4*128] so the rhs slice is 2D
            wv_sb = constp.tile([P, CK, 2, 4 * P], BF16)
            for c in range(CK):
                nc.sync.dma_start(
                    wv_sb[:, c, :, :],
                    wv[:, c, :, :].rearrange("p (g r) f -> p g (r f)", g=2),
                )

            # [s_p, head-in-group, s_tile, 64 v cols + 1 ones col]; one tile
            # per pair-group so AV(0..3) never depends on vproj(1)
            v_all = [constp.tile([P, 8, TT, HS + 1], BF16, name=f"vall{g}")
                     for g in range(2)]
            for g in range(2):
                nc.gpsimd.memset(v_all[g][:, :, :, HS:HS + 1], 1.0)

            # ---- helpers ----
            def proj(pair):
                """qT, kT [128(dims of 2 heads), T] bf16 for this pair."""
                wq_sb, wk_sb = w_sb.pop(pair)
                qT = qkp.tile([P, T], BF16, tag="qk", name=f"q{pair}")
                kT = qkp.tile([P, T], BF16, tag="qk", name=f"k{pair}")
                for g in range(2):
                    for wsb, dst in ((wq_sb, qT), (wk_sb, kT)):
                        pp = psA.tile([P, HALF], F32, tag="ps",
                                      name=f"pp{pair}_{g}")
                        for c in range(CK):
                            nc.tensor.matmul(
                                pp[:, :],
                                wsb[:, c, :],
                                xt_sb[c][:, ds(HALF * g, HALF)],
                                start=(c == 0),
                                stop=(c == CK - 1),
                            )
                        nc.vector.tensor_copy(dst[:, ds(HALF * g, HALF)], pp[:, :])
                return qT, kT

            def scores(pair, qT, kT):
                """es tiles [128, s-tile, head, 512] per t-pass, exp'd + masked.

                Pass A (t < 512) emitted first so AV(hh=0) of this pair only
                waits ~3us of ScalarE exp, not the full ~11us."""
                esA = espA.tile([P, 4, 2, HALF], BF16, tag="esA", name=f"eA{pair}")
                esB = espB.tile([P, TT, 2, HALF], BF16, tag="esB", name=f"eB{pair}")
                for i in range(4):
                    # pass A: t in [t0, 512)  (only s-tiles in the first half)
                    t0 = P * i
                    wa = HALF - t0
                    sc = psSc.tile([P, 2, HALF], F32, tag="sc",
                                   name=f"scA{pair}_{i}")
                    for w in range(2):
                        nc.tensor.matmul(
                            sc[:, w, ds(t0, wa)],
                            kT[ds(HS * w, HS), ds(t0, P)],
                            qT[ds(HS * w, HS), ds(t0, wa)],
                        )
                    nc.scalar.activation(
                        esA[:, i, :, ds(t0, wa)],
                        sc[:, :, ds(t0, wa)],
                        mybir.ActivationFunctionType.Exp,
                        scale=HS ** -0.5,
                    )
                    nc.vector.tensor_tensor(
                        esA[:, i, :, ds(t0, P)], esA[:, i, :, ds(t0, P)],
                        maskrep[:, :, :], mybir.AluOpType.mult,
                    )
                for i in range(TT):
                    # pass B: t in [max(t0,512), 1024)
                    t0 = P * i
                    a0 = max(t0, HALF)
                    wb = T - a0
                    lb = a0 - HALF
                    sc = psSc.tile([P, 2, HALF], F32, tag="sc",
                                   name=f"scB{pair}_{i}")
                    for w in range(2):
                        nc.tensor.matmul(
                            sc[:, w, ds(lb, wb)],
                            kT[ds(HS * w, HS), ds(P * i, P)],
                            qT[ds(HS * w, HS), ds(a0, wb)],
                        )
                    nc.scalar.activation(
                        esB[:, i, :, ds(lb, wb)],
                        sc[:, :, ds(lb, wb)],
                        mybir.ActivationFunctionType.Exp,
                        scale=HS ** -0.5,
                    )
                    if i >= 4:
                        nc.vector.tensor_tensor(
                            esB[:, i, :, ds(lb, P)], esB[:, i, :, ds(lb, P)],
                            maskrep[:, :, :], mybir.AluOpType.mult,
                        )
                return esA, esB

            def attn_v(pair, esA, esB, late=False):
                """out^T accumulation + normalize + DMA for both heads.

                DVE reciprocal cost scales with per-lane free size, so the
                [1, 1024] denominator row is DMA-repartitioned to [128, 8],
                recip'd there, and DMA'd back before the GpSimd broadcast."""
                for w in range(2):
                    h = 2 * pair + w
                    vt = v_all[pair // 4]
                    hg = h - 8 * (pair // 4)
                    avs = normp.tile([HS + 1, T], BF16, tag="avs",
                                     name=f"avs{h}")
                    for hh in range(2):
                        # the very last pair runs after exp(7) released the
                        # psSc banks; borrowing them doubles the av rotation
                        avp = psSc if (late and hh == w) else psV
                        av = avp.tile([HS + 1, HALF], F32,
                                      tag=("sc" if avp is psSc else "av"),
                                      name=f"av{h}_{hh}")
                        if hh == 0:
                            contrib = [(i, esA, P * i, HALF - P * i)
                                       for i in range(4)]
                        else:
                            contrib = [(i, esB, max(P * i, HALF) - HALF,
                                        T - max(P * i, HALF))
                                       for i in range(TT)]
                        for idx, (i, est, lo, wd) in enumerate(contrib):
                            nc.tensor.matmul(
                                av[:, ds(lo, wd)],
                                vt[:, hg, i, :],
                                est[:, i, w, ds(lo, wd)],
                                start=(idx == 0),
                                stop=(idx == len(contrib) - 1),
                            )
                        nc.vector.tensor_copy(
                            avs[:, ds(HALF * hh, HALF)], av[:, :])
                    _norm_chain(h, avs, 0, T, f"{h}")

            def _norm_chain(h, avs, off, width, tag_sfx):
                den_t = normp.tile([P, TT], BF16, tag="dent",
                                   name=f"dt{tag_sfx}")
                nw = width // P
                nc.sync.dma_start(den_t[:, 0:nw], avs[HS:HS + 1, ds(off, width)])
                rec_t = normp.tile([P, TT], BF16, tag="rect",
                                   name=f"rt{tag_sfx}")
                with nc.allow_low_precision(
                        reason="bf16 softmax denom: ~0.4% rel err, "
                        "within the 2e-2 gate"):
                    nc.vector.reciprocal(rec_t[:, 0:nw], den_t[:, 0:nw])
                rec = normp.tile([1, T], BF16, tag="rec", name=f"rec{tag_sfx}")
                nc.sync.dma_start(rec[:, ds(off, width)], rec_t[:, 0:nw])
                rb = normp.tile([HS, T], BF16, tag="rb", name=f"rb{tag_sfx}")
                nc.gpsimd.partition_broadcast(
                    rb[:, ds(off, width)], rec[0:1, ds(off, width)])
                osb = normp.tile([HS, T], BF16, tag="osb", name=f"osb{tag_sfx}")
                nc.vector.tensor_tensor(
                    osb[:, ds(off, width)], avs[0:HS, ds(off, width)],
                    rb[:, ds(off, width)], mybir.AluOpType.mult,
                )
                nc.sync.dma_start(out[h, :, ds(off, width)],
                                  osb[:, ds(off, width)])

            # ---- v for one pair-group (8 heads), split by s-tile half ----
            def vproj(pg, jlo, jhi):
                for j in range(jlo, jhi):
                    pv = psA.tile([P, HALF], F32, tag="ps", name=f"pv{j}_{pg}")
                    for c in range(CK):
                        nc.tensor.matmul(
                            pv[:, :],
                            xt_sb[c][:, ts(j, P)],
                            wv_sb[:, c, pg, :],
                            start=(c == 0),
                            stop=(c == CK - 1),
                        )
                    # pv cols are (head0..head7 of group) x 64 in order
                    nc.vector.tensor_copy(
                        v_all[pg][:, :, j, 0:HS],
                        pv.rearrange("p (g d) -> p g d", d=HS),
                    )

            # ---- software-pipelined pair loop ----
            # v3-like interleave: AV(p-1) right after scores(p) so each
            # pair's normalize chain (DVE/DMA/GpSimd) hides under the next
            # projection.  The endgame keeps AV(5..7) (~11.4us of PE work)
            # after scores(7) to exactly cover exp(7) on ScalarE; only the
            # last head's normalize chain is exposed as tail.
            es = {}
            qk1 = proj(0)
            es[0] = scores(0, *qk1)
            qk1 = proj(1)
            es[1] = scores(1, *qk1)
            vproj(0, 0, TT)
            attn_v(0, *es.pop(0))
            w_sb[4] = dma_w(4)
            qk1 = proj(2)
            es[2] = scores(2, *qk1)
            attn_v(1, *es.pop(1))
            vproj(1, 0, 4)
            w_sb[5] = dma_w(5)
            qk1 = proj(3)
            es[3] = scores(3, *qk1)
            attn_v(2, *es.pop(2))
            vproj(1, 4, TT)
            w_sb[6] = dma_w(6)
            qk1 = proj(4)
            es[4] = scores(4, *qk1)
            attn_v(3, *es.pop(3))
            w_sb[7] = dma_w(7)
            qk1 = proj(5)
            es[5] = scores(5, *qk1)
            attn_v(4, *es.pop(4))
            qk1 = proj(6)
            es[6] = scores(6, *qk1)
            qk1 = proj(7)
            es[7] = scores(7, *qk1)
            attn_v(5, *es.pop(5))
            attn_v(6, *es.pop(6))
            attn_v(7, *es.pop(7), late=True)
    nc.compile()
    return nc


def get_nc():
    global _BUILT
    if _BUILT is None:
        _BUILT = build_nc()
    return _BUILT


def prep_inputs(x, Wq, Wk, Wv):
    """Host-side shard + layout prep. Returns in_maps (one dict per core)."""
    x = np.asarray(x, dtype=np.float32)
    Wq = np.asarray(Wq, dtype=np.float32)
    Wk = np.asarray(Wk, dtype=np.float32)
    Wv = np.asarray(Wv, dtype=np.float32)
    bf = ml_dtypes.bfloat16

    # xT[b]: [C, T] -> [p, c, t] with row 128c+p
    xts = []
    for b in range(B):
        xT = np.ascontiguousarray(x[b].T)          # [C, T]
        xts.append(xT.reshape(CK, P, T).transpose(1, 0, 2).astype(bf))

    def pack_pairs(W):
        # [H, C, hs] -> [pair, C, 128] -> [pair, p, c, f]
        Wp = W.reshape(PAIRS, 2, C, HS).transpose(0, 2, 1, 3).reshape(PAIRS, C, P)
        return Wp.reshape(PAIRS, CK, P, P).transpose(0, 2, 1, 3)  # [pair, p, c, f]

    wq_p = pack_pairs(Wq)
    wk_p = pack_pairs(Wk)
    wqk_host = np.stack([wq_p, wk_p], axis=0).astype(bf)  # [2, pair, p, c, f]
    # wv: [p, c, pair, f]
    wv_host = np.ascontiguousarray(pack_pairs(Wv).transpose(1, 2, 0, 3)).astype(bf)

    return [
        {"xt": np.ascontiguousarray(xts[b]), "wqk": wqk_host, "wv": wv_host}
        for b in range(B)
    ]


def run_on_device(in_maps, **kwargs):
    nc = get_nc()
    return run_bass_kernel_spmd(nc, in_maps, list(range(B)), **kwargs)


def assemble(core_out):
    """[H, HS, T] out^T -> [T, H*HS]: pure layout transpose."""
    o = np.asarray(core_out, dtype=np.float32)
    return np.ascontiguousarray(o.transpose(2, 0, 1).reshape(T, H * HS))


def kernel(x, Wq, Wk, Wv):
    in_maps = prep_inputs(x, Wq, Wk, Wv)
    res = run_on_device(in_maps)
    return np.stack([assemble(res.results[b]["out"]) for b in range(B)], axis=0)


# revision 30
# speedup vs baseline: 1.0200x; 1.0200x over previous
"""Multi-head causal attention (B=8, T=1024, C=1024, H=16, hs=64) on 8 trn2 cores.

Data-parallel over batch: core b computes full attention for x[b].

Device algorithm (per core), matmuls bf16 inputs / fp32 PSUM accum:
  - xT [C, T] resident in SBUF (host pre-transposed, bf16); ~24 dummy
    matmuls at t=0 warm the PE HAM clock gate during the input DMAs.
  - software-pipelined pair loop so ScalarE exp work (~90us, the
    secondary critical engine) starts at ~12us and overlaps the PE
    stream; AV(p-1) interleaves between scores(p) and proj(p+1) so each
    pair's normalize chain hides under the next projection; the tail
    keeps AV(5..7) (~11us of PE work) after scores(7) to cover exp(7).
  - scores computed transposed (scT[s, t]) per head-pair in two t-passes
    of 512 cols: PSUM tile [128, 2(head), 512] per (s-tile, pass); the two
    heads' K=64 matmuls run concurrently in PE row groups (0,0)/(64,0)
    into different PSUM banks; ONE exp per tile on ScalarE (free AP
    [2, width], scale=1/8) -> mega es tiles [128, s-tile, head, 512].
  - causal diagonal 128x128 blocks masked by one tril multiply per
    (s-tile): [128, 2, 128] DVE op against a replicated tril.
  - out^T[65, t] accumulated over s chunks: lhsT = [v | 1], rhs = expT.
    Row 64 = sum(exp) = softmax denominator.
  - normalize per (pair, head): copy both 512-halves into avs [65, 1024],
    DVE reciprocal of the denom row, GpSimd partition-broadcast, DVE
    multiply -> osb bf16, single 128KB DMA to out[h].
"""

import numpy as np
import ml_dtypes

import concourse.bass as bass
import concourse.mybir as mybir
from concourse import bacc
from concourse.tile import TileContext
from concourse.bass import ds, ts
from concourse.bass_utils import run_bass_kernel_spmd
from concourse.masks import make_upper_triangular

BF16 = mybir.dt.bfloat16
F32 = mybir.dt.float32

B, T, C, H, HS = 8, 1024, 1024, 16, 64
P = 128
CK = C // P       # 8 contraction chunks
TT = T // P       # 8 s tiles
PAIRS = H // 2    # 8 head pairs
HALF = 512

_BUILT = None


def build_nc():
    nc = bacc.Bacc("TRN2", target_bir_lowering=False, debug=False)
    # [p, c, t] : xT[C, T] chunked; partition p, chunk c -> row 128c+p of xT
    xt = nc.dram_tensor("xt", [P, CK, T], BF16, kind="ExternalInput")
    # [proj(q,k), pair, p, c, f] : lhsT chunks, f = 2 heads x 64 stacked
    wqk = nc.dram_tensor("wqk", [2, PAIRS, P, CK, P], BF16, kind="ExternalInput")
    # [p, c, pair, f]
    wv = nc.dram_tensor("wv", [P, CK, PAIRS, P], BF16, kind="ExternalInput")
    # out^T per head: [head, d, t]; host transposes to [T, H*HS]
    out = nc.dram_tensor("out", [H, HS, T], BF16, kind="ExternalOutput")

    with TileContext(nc) as tc:
        with (
            tc.tile_pool(name="const", bufs=1) as constp,
            tc.tile_pool(name="wpool", bufs=6) as wpool,
            tc.tile_pool(name="qkpool", bufs=6) as qkp,
            tc.tile_pool(name="espool", bufs=4) as espA,
            tc.tile_pool(name="espoolB", bufs=4) as espB,
            tc.tile_pool(name="normpool", bufs=2) as normp,
            tc.tile_pool(name="psA", bufs=2, space="PSUM") as psA,
            tc.tile_pool(name="psSc", bufs=2, space="PSUM") as psSc,
            tc.tile_pool(name="psV", bufs=2, space="PSUM") as psV,
        ):
            # ---- PE warm-up + ScalarE table preload (run during DMA-in) ----
            maskrep = constp.tile([P, 2, P], BF16)
            for j in range(2):
                make_upper_triangular(nc, maskrep[:, j, :], val=1.0, diag=True)
            # ~24 trivial matmuls keep the PE busy >3.4us so the HAM clock
            # gate opens before the real stream starts (else the first
            # ~20us of projections run at 1.2GHz).
            dummy = psV.tile([HS + 1, HALF], F32, tag="av", name="warmup")
            for r in range(24):
                nc.tensor.matmul(
                    dummy[0:1, 0:2 * P],
                    maskrep[:, 0, 0:1],
                    maskrep[:, :, :],
                )
            # warm the ScalarE exp table during the input-DMA window
            scratch = constp.tile([1, 8], BF16)
            nc.gpsimd.memset(scratch[:, :], 0.0)
            nc.scalar.activation(scratch[:, :], scratch[:, :],
                                 mybir.ActivationFunctionType.Exp)

            def dma_w(pair):
                wq_sb = wpool.tile([P, CK, P], BF16, tag="w", name=f"wq{pair}")
                nc.sync.dma_start(wq_sb[:, :, :], wqk[0, pair, :, :, :])
                wk_sb = wpool.tile([P, CK, P], BF16, tag="w", name=f"wk{pair}")
                nc.sync.dma_start(wk_sb[:, :, :], wqk[1, pair, :, :, :])
                return wq_sb, wk_sb

            # pair-0 weights first (small), then xt chunk-by-chunk in
            # separate tiles so proj(0)'s c=0 matmul starts on chunk 0.
            w_sb = {0: dma_w(0)}
            xt_sb = [constp.tile([P, T], BF16, name=f"xt{c}") for c in range(CK)]
            for c in range(CK):
                nc.sync.dma_start(xt_sb[c][:, :], xt[:, c, :])
            w_sb[1] = dma_w(1)
            w_sb[2] = dma_w(2)
            w_sb[3] = dma_w(3)

            # pair-group-major: [p, c, pg, 4*128] so the rhs slice is 2D
            wv_sb = constp.tile([P, CK, 2, 4 * P], BF16)
            for c in range(CK):
                nc.sync.dma_start(
                    wv_sb[:, c, :, :],
                    wv[:, c, :, :].rearrange("p (g r) f -> p g (r f)", g=2),
                )

            # [s_p, head-in-group, s_tile, 64 v cols + 1 ones col]; one tile
            # per pair-group so AV(0..3) never depends on vproj(1)
            v_all = [constp.tile([P, 8, TT, HS + 1], BF16, name=f"vall{g}")
                     for g in range(2)]
            for g in range(2):
                nc.gpsimd.memset(v_all[g][:, :, :, HS:HS + 1], 1.0)

            # ---- helpers ----
            def proj(pair, first=False):
                """qT, kT [128(dims of 2 heads), T] bf16 for this pair.

                first=True (pair 0 only): the xt chunks are still landing
                ~640ns apart, so run all 4 accumulation chains chunk-major
                (852ns of matmul per chunk) to stay ahead of the DMA; the
                two extra PSUM tiles borrow the still-empty psSc pool."""
                wq_sb, wk_sb = w_sb.pop(pair)
                qT = qkp.tile([P, T], BF16, tag="qk", name=f"q{pair}")
                kT = qkp.tile([P, T], BF16, tag="qk", name=f"k{pair}")
                chains = [(g, wsb, dst)
                          for g in range(2) for wsb, dst in ((wq_sb, qT),
                                                             (wk_sb, kT))]
                if first:
                    pps = [
                        (psA if g == 0 else psSc).tile(
                            [P, HALF], F32,
                            tag=("ps" if g == 0 else "sc"),
                            name=f"pp{pair}_{g}_{0 if dst is qT else 1}")
                        for g, wsb, dst in chains]
                    for c in range(CK):
                        for ci, (g, wsb, dst) in enumerate(chains):
                            nc.tensor.matmul(
                                pps[ci][:, :],
                                wsb[:, c, :],
                                xt_sb[c][:, ds(HALF * g, HALF)],
                                start=(c == 0),
                                stop=(c == CK - 1),
                            )
                    for ci, (g, wsb, dst) in enumerate(chains):
                        nc.vector.tensor_copy(dst[:, ds(HALF * g, HALF)],
                                              pps[ci][:, :])
                    return qT, kT
                for g, wsb, dst in chains:
                    pp = psA.tile([P, HALF], F32, tag="ps",
                                  name=f"pp{pair}_{g}")
                    for c in range(CK):
                        nc.tensor.matmul(
                            pp[:, :],
                            wsb[:, c, :],
                            xt_sb[c][:, ds(HALF * g, HALF)],
                            start=(c == 0),
                            stop=(c == CK - 1),
                        )
                    nc.vector.tensor_copy(dst[:, ds(HALF * g, HALF)], pp[:, :])
                return qT, kT

            def scores(pair, qT, kT):
                """es tiles [128, s-tile, head, 512] per t-pass, exp'd + masked.

                Pass A (t < 512) emitted first so AV(hh=0) of this pair only
                waits ~3us of ScalarE exp, not the full ~11us."""
                esA = espA.tile([P, 4, 2, HALF], BF16, tag="esA", name=f"eA{pair}")
                esB = espB.tile([P, TT, 2, HALF], BF16, tag="esB", name=f"eB{pair}")
                for i in range(4):
                    # pass A: t in [t0, 512)  (only s-tiles in the first half)
                    t0 = P * i
                    wa = HALF - t0
                    sc = psSc.tile([P, 2, HALF], F32, tag="sc",
                                   name=f"scA{pair}_{i}")
                    for w in range(2):
                        nc.tensor.matmul(
                            sc[:, w, ds(t0, wa)],
                            kT[ds(HS * w, HS), ds(t0, P)],
                            qT[ds(HS * w, HS), ds(t0, wa)],
                        )
                    nc.scalar.activation(
                        esA[:, i, :, ds(t0, wa)],
                        sc[:, :, ds(t0, wa)],
                        mybir.ActivationFunctionType.Exp,
                        scale=HS ** -0.5,
                    )
                    nc.vector.tensor_tensor(
                        esA[:, i, :, ds(t0, P)], esA[:, i, :, ds(t0, P)],
                        maskrep[:, :, :], mybir.AluOpType.mult,
                    )
                for i in range(TT):
                    # pass B: t in [max(t0,512), 1024)
                    t0 = P * i
                    a0 = max(t0, HALF)
                    wb = T - a0
                    lb = a0 - HALF
                    sc = psSc.tile([P, 2, HALF], F32, tag="sc",
                                   name=f"scB{pair}_{i}")
                    for w in range(2):
                        nc.tensor.matmul(
                            sc[:, w, ds(lb, wb)],
                            kT[ds(HS * w, HS), ds(P * i, P)],
                            qT[ds(HS * w, HS), ds(a0, wb)],
                        )
                    nc.scalar.activation(
                        esB[:, i, :, ds(lb, wb)],
                        sc[:, :, ds(lb, wb)],
                        mybir.ActivationFunctionType.Exp,
                        scale=HS ** -0.5,
                    )
                    if i >= 4:
                        nc.vector.tensor_tensor(
                            esB[:, i, :, ds(lb, P)], esB[:, i, :, ds(lb, P)],
                            maskrep[:, :, :], mybir.AluOpType.mult,
                        )
                return esA, esB

            def attn_v(pair, esA, esB, late=False):
                """out^T accumulation + normalize + DMA for both heads.

                DVE reciprocal cost scales with per-lane free size, so the
                [1, 1024] denominator row is DMA-repartitioned to [128, 8],
                recip'd there, and DMA'd back before the GpSimd broadcast."""
                for w in range(2):
                    h = 2 * pair + w
                    vt = v_all[pair // 4]
                    hg = h - 8 * (pair // 4)
                    avs = normp.tile([HS + 1, T], BF16, tag="avs",
                                     name=f"avs{h}")
                    for hh in range(2):
                        # the very last pair runs after exp(7) released the
                        # psSc banks; borrowing them doubles the av rotation
                        avp = psSc if (late and hh == w) else psV
                        av = avp.tile([HS + 1, HALF], F32,
                                      tag=("sc" if avp is psSc else "av"),
                                      name=f"av{h}_{hh}")
                        if hh == 0:
                            contrib = [(i, esA, P * i, HALF - P * i)
                                       for i in range(4)]
                        else:
                            contrib = [(i, esB, max(P * i, HALF) - HALF,
                                        T - max(P * i, HALF))
                                       for i in range(TT)]
                        for idx, (i, est, lo, wd) in enumerate(contrib):
                            nc.tensor.matmul(
                                av[:, ds(lo, wd)],
                                vt[:, hg, i, :],
                                est[:, i, w, ds(lo, wd)],
                                start=(idx == 0),
                                stop=(idx == len(contrib) - 1),
                            )
                        nc.vector.tensor_copy(
                            avs[:, ds(HALF * hh, HALF)], av[:, :])
                    _norm_chain(h, avs, 0, T, f"{h}")

            def _norm_chain(h, avs, off, width, tag_sfx):
                den_t = normp.tile([P, TT], BF16, tag="dent",
                                   name=f"dt{tag_sfx}")
                nw = width // P
                nc.sync.dma_start(den_t[:, 0:nw], avs[HS:HS + 1, ds(off, width)])
                rec_t = normp.tile([P, TT], BF16, tag="rect",
                                   name=f"rt{tag_sfx}")
                with nc.allow_low_precision(
                        reason="bf16 softmax denom: ~0.4% rel err, "
                        "within the 2e-2 gate"):
                    nc.vector.reciprocal(rec_t[:, 0:nw], den_t[:, 0:nw])
                rec = normp.tile([1, T], BF16, tag="rec", name=f"rec{tag_sfx}")
                nc.sync.dma_start(rec[:, ds(off, width)], rec_t[:, 0:nw])
                rb = normp.tile([HS, T], BF16, tag="rb", name=f"rb{tag_sfx}")
                nc.gpsimd.partition_broadcast(
                    rb[:, ds(off, width)], rec[0:1, ds(off, width)])
                osb = normp.tile([HS, T], BF16, tag="osb", name=f"osb{tag_sfx}")
                nc.vector.tensor_tensor(
                    osb[:, ds(off, width)], avs[0:HS, ds(off, width)],
                    rb[:, ds(off, width)], mybir.AluOpType.mult,
                )
                nc.sync.dma_start(out[h, :, ds(off, width)],
                                  osb[:, ds(off, width)])

            # ---- v for one pair-group (8 heads), split by s-tile half ----
            def vproj(pg, jlo, jhi):
                for j in range(jlo, jhi):
                    pv = psA.tile([P, HALF], F32, tag="ps", name=f"pv{j}_{pg}")
                    for c in range(CK):
                        nc.tensor.matmul(
                            pv[:, :],
                            xt_sb[c][:, ts(j, P)],
                            wv_sb[:, c, pg, :],
                            start=(c == 0),
                            stop=(c == CK - 1),
                        )
                    # pv cols are (head0..head7 of group) x 64 in order
                    nc.vector.tensor_copy(
                        v_all[pg][:, :, j, 0:HS],
                        pv.rearrange("p (g d) -> p g d", d=HS),
                    )

            # ---- software-pipelined pair loop ----
            # v3-like interleave: AV(p-1) right after scores(p) so each
            # pair's normalize chain (DVE/DMA/GpSimd) hides under the next
            # projection.  The endgame keeps AV(5..7) (~11.4us of PE work)
            # after scores(7) to exactly cover exp(7) on ScalarE; only the
            # last head's normalize chain is exposed as tail.
            es = {}
            qk1 = proj(0, first=True)
            es[0] = scores(0, *qk1)
            qk1 = proj(1)
            es[1] = scores(1, *qk1)
            vproj(0, 0, TT)
            attn_v(0, *es.pop(0))
            w_sb[4] = dma_w(4)
            qk1 = proj(2)
            es[2] = scores(2, *qk1)
            attn_v(1, *es.pop(1))
            vproj(1, 0, 4)
            w_sb[5] = dma_w(5)
            qk1 = proj(3)
            es[3] = scores(3, *qk1)
            attn_v(2, *es.pop(2))
            vproj(1, 4, TT)
            w_sb[6] = dma_w(6)
            qk1 = proj(4)
            es[4] = scores(4, *qk1)
            attn_v(3, *es.pop(3))
            w_sb[7] = dma_w(7)
            qk1 = proj(5)
            es[5] = scores(5, *qk1)
            attn_v(4, *es.pop(4))
            qk1 = proj(6)
            es[6] = scores(6, *qk1)
            qk1 = proj(7)
            es[7] = scores(7, *qk1)
            attn_v(5, *es.pop(5))
            attn_v(6, *es.pop(6))
            attn_v(7, *es.pop(7), late=True)
    nc.compile()
    return nc


def get_nc():
    global _BUILT
    if _BUILT is None:
        _BUILT = build_nc()
    return _BUILT


def prep_inputs(x, Wq, Wk, Wv):
    """Host-side shard + layout prep. Returns in_maps (one dict per core)."""
    x = np.asarray(x, dtype=np.float32)
    Wq = np.asarray(Wq, dtype=np.float32)
    Wk = np.asarray(Wk, dtype=np.float32)
    Wv = np.asarray(Wv, dtype=np.float32)
    bf = ml_dtypes.bfloat16

    # xT[b]: [C, T] -> [p, c, t] with row 128c+p
    xts = []
    for b in range(B):
        xT = np.ascontiguousarray(x[b].T)          # [C, T]
        xts.append(xT.reshape(CK, P, T).transpose(1, 0, 2).astype(bf))

    def pack_pairs(W):
        # [H, C, hs] -> [pair, C, 128] -> [pair, p, c, f]
        Wp = W.reshape(PAIRS, 2, C, HS).transpose(0, 2, 1, 3).reshape(PAIRS, C, P)
        return Wp.reshape(PAIRS, CK, P, P).transpose(0, 2, 1, 3)  # [pair, p, c, f]

    wq_p = pack_pairs(Wq)
    wk_p = pack_pairs(Wk)
    wqk_host = np.stack([wq_p, wk_p], axis=0).astype(bf)  # [2, pair, p, c, f]
    # wv: [p, c, pair, f]
    wv_host = np.ascontiguousarray(pack_pairs(Wv).transpose(1, 2, 0, 3)).astype(bf)

    return [
        {"xt": np.ascontiguousarray(xts[b]), "wqk": wqk_host, "wv": wv_host}
        for b in range(B)
    ]


def run_on_device(in_maps, **kwargs):
    nc = get_nc()
    return run_bass_kernel_spmd(nc, in_maps, list(range(B)), **kwargs)


def assemble(core_out):
    """[H, HS, T] out^T -> [T, H*HS]: pure layout transpose."""
    o = np.asarray(core_out, dtype=np.float32)
    return np.ascontiguousarray(o.transpose(2, 0, 1).reshape(T, H * HS))


def kernel(x, Wq, Wk, Wv):
    in_maps = prep_inputs(x, Wq, Wk, Wv)
    res = run_on_device(in_maps)
    return np.stack([assemble(res.results[b]["out"]) for b in range(B)], axis=0)


# revision 33
# speedup vs baseline: 1.0280x; 1.0079x over previous
"""Multi-head causal attention (B=8, T=1024, C=1024, H=16, hs=64) on 8 trn2 cores.

Data-parallel over batch: core b computes full attention for x[b].

Device algorithm (per core), matmuls bf16 inputs / fp32 PSUM accum:
  - xT [C, T] resident in SBUF (host pre-transposed, bf16); ~24 dummy
    matmuls at t=0 warm the PE HAM clock gate during the input DMAs.
  - software-pipelined pair loop so ScalarE exp work (~90us, the
    secondary critical engine) starts at ~12us and overlaps the PE
    stream; AV(p-1) interleaves between scores(p) and proj(p+1) so each
    pair's normalize chain hides under the next projection; the tail
    keeps AV(5..7) (~11us of PE work) after scores(7) to cover exp(7).
  - scores computed transposed (scT[s, t]) per head-pair in two t-passes
    of 512 cols: PSUM tile [128, 2(head), 512] per (s-tile, pass); the two
    heads' K=64 matmuls run concurrently in PE row groups (0,0)/(64,0)
    into different PSUM banks; ONE exp per tile on ScalarE (free AP
    [2, width], scale=1/8) -> mega es tiles [128, s-tile, head, 512].
  - causal diagonal 128x128 blocks masked by one tril multiply per
    (s-tile): [128, 2, 128] DVE op against a replicated tril.
  - out^T[65, t] accumulated over s chunks: lhsT = [v | 1], rhs = expT.
    Row 64 = sum(exp) = softmax denominator.
  - normalize per (pair, head): copy both 512-halves into avs [65, 1024],
    DVE reciprocal of the denom row, GpSimd partition-broadcast, DVE
    multiply -> osb bf16, single 128KB DMA to out[h].
"""

import numpy as np
import ml_dtypes

import concourse.bass as bass
import concourse.mybir as mybir
from concourse import bacc
from concourse.tile import TileContext
from concourse.bass import ds, ts
from concourse.bass_utils import run_bass_kernel_spmd
from concourse.masks import make_upper_triangular

BF16 = mybir.dt.bfloat16
F32 = mybir.dt.float32

B, T, C, H, HS = 8, 1024, 1024, 16, 64
P = 128
CK = C // P       # 8 contraction chunks
TT = T // P       # 8 s tiles
PAIRS = H // 2    # 8 head pairs
HALF = 512

_BUILT = None


def _act_recip(nc, out_ap, in_ap):
    """ACT-engine reciprocal (LUT): ~1e-5 rel err, fast on [1, N] rows.

    Only used for the tail pairs, after the exp stream is done: recip
    lives in a different ACT table set than exp (one ~2.7us switch on the
    by-then-idle ScalarE, never switched back)."""
    ins = [nc.scalar.lower_ap(in_ap),
           mybir.ImmediateValue(dtype=mybir.dt.float32, value=0.0),
           mybir.ImmediateValue(dtype=mybir.dt.float32, value=1.0),
           mybir.ImmediateValue(dtype=mybir.dt.float32, value=0.0)]
    outs = [nc.scalar.lower_ap(out_ap)]
    return nc.scalar.add_instruction(
        mybir.InstActivation(
            name=nc.get_next_instruction_name(),
            func=mybir.ActivationFunctionType.Reciprocal,
            ins=ins,
            outs=outs,
        ))


def build_nc():
    nc = bacc.Bacc("TRN2", target_bir_lowering=False, debug=False)
    # [p, c, t] : xT[C, T] chunked; partition p, chunk c -> row 128c+p of xT
    xt = nc.dram_tensor("xt", [P, CK, T], BF16, kind="ExternalInput")
    # [proj(q,k), pair, p, c, f] : lhsT chunks, f = 2 heads x 64 stacked
    wqk = nc.dram_tensor("wqk", [2, PAIRS, P, CK, P], BF16, kind="ExternalInput")
    # [p, c, pair, f]
    wv = nc.dram_tensor("wv", [P, CK, PAIRS, P], BF16, kind="ExternalInput")
    # out^T per head: [head, d, t]; host transposes to [T, H*HS]
    out = nc.dram_tensor("out", [H, HS, T], BF16, kind="ExternalOutput")

    with TileContext(nc) as tc:
        with (
            tc.tile_pool(name="const", bufs=1) as constp,
            tc.tile_pool(name="wpool", bufs=6) as wpool,
            tc.tile_pool(name="qkpool", bufs=6) as qkp,
            tc.tile_pool(name="espool", bufs=4) as espA,
            tc.tile_pool(name="espoolB", bufs=4) as espB,
            tc.tile_pool(name="normpool", bufs=2) as normp,
            tc.tile_pool(name="psA", bufs=2, space="PSUM") as psA,
            tc.tile_pool(name="psSc", bufs=2, space="PSUM") as psSc,
            tc.tile_pool(name="psV", bufs=2, space="PSUM") as psV,
        ):
            # ---- PE warm-up + ScalarE table preload (run during DMA-in) ----
            maskrep = constp.tile([P, 2, P], BF16)
            for j in range(2):
                make_upper_triangular(nc, maskrep[:, j, :], val=1.0, diag=True)
            # ~24 trivial matmuls keep the PE busy >3.4us so the HAM clock
            # gate opens before the real stream starts (else the first
            # ~20us of projections run at 1.2GHz).
            dummy = psV.tile([HS + 1, HALF], F32, tag="av", name="warmup")
            for r in range(24):
                nc.tensor.matmul(
                    dummy[0:1, 0:2 * P],
                    maskrep[:, 0, 0:1],
                    maskrep[:, :, :],
                )
            # warm the ScalarE exp table during the input-DMA window
            scratch = constp.tile([1, 8], BF16)
            nc.gpsimd.memset(scratch[:, :], 0.0)
            nc.scalar.activation(scratch[:, :], scratch[:, :],
                                 mybir.ActivationFunctionType.Exp)

            def dma_w(pair):
                wq_sb = wpool.tile([P, CK, P], BF16, tag="w", name=f"wq{pair}")
                nc.sync.dma_start(wq_sb[:, :, :], wqk[0, pair, :, :, :])
                wk_sb = wpool.tile([P, CK, P], BF16, tag="w", name=f"wk{pair}")
                nc.sync.dma_start(wk_sb[:, :, :], wqk[1, pair, :, :, :])
                return wq_sb, wk_sb

            # pair-0 weights first (small), then xt chunk-by-chunk in
            # separate tiles so proj(0)'s c=0 matmul starts on chunk 0.
            w_sb = {0: dma_w(0)}
            xt_sb = [constp.tile([P, T], BF16, name=f"xt{c}") for c in range(CK)]
            for c in range(CK):
                nc.sync.dma_start(xt_sb[c][:, :], xt[:, c, :])
            w_sb[1] = dma_w(1)
            w_sb[2] = dma_w(2)
            w_sb[3] = dma_w(3)

            # pair-group-major: [p, c, pg, 4*128] so the rhs slice is 2D
            wv_sb = constp.tile([P, CK, 2, 4 * P], BF16)
            for c in range(CK):
                nc.sync.dma_start(
                    wv_sb[:, c, :, :],
                    wv[:, c, :, :].rearrange("p (g r) f -> p g (r f)", g=2),
                )

            # [s_p, head-in-group, s_tile, 64 v cols + 1 ones col]; one tile
            # per pair-group so AV(0..3) never depends on vproj(1)
            v_all = [constp.tile([P, 8, TT, HS + 1], BF16, name=f"vall{g}")
                     for g in range(2)]
            for g in range(2):
                nc.gpsimd.memset(v_all[g][:, :, :, HS:HS + 1], 1.0)

            # ---- helpers ----
            def proj(pair, first=False):
                """qT, kT [128(dims of 2 heads), T] bf16 for this pair.

                first=True (pair 0 only): the xt chunks are still landing
                ~640ns apart, so run all 4 accumulation chains chunk-major
                (852ns of matmul per chunk) to stay ahead of the DMA; the
                two extra PSUM tiles borrow the still-empty psSc pool."""
                wq_sb, wk_sb = w_sb.pop(pair)
                qT = qkp.tile([P, T], BF16, tag="qk", name=f"q{pair}")
                kT = qkp.tile([P, T], BF16, tag="qk", name=f"k{pair}")
                chains = [(g, wsb, dst)
                          for g in range(2) for wsb, dst in ((wq_sb, qT),
                                                             (wk_sb, kT))]
                if first:
                    pps = [
                        (psA if g == 0 else psSc).tile(
                            [P, HALF], F32,
                            tag=("ps" if g == 0 else "sc"),
                            name=f"pp{pair}_{g}_{0 if dst is qT else 1}")
                        for g, wsb, dst in chains]
                    for c in range(CK):
                        for ci, (g, wsb, dst) in enumerate(chains):
                            nc.tensor.matmul(
                                pps[ci][:, :],
                                wsb[:, c, :],
                                xt_sb[c][:, ds(HALF * g, HALF)],
                                start=(c == 0),
                                stop=(c == CK - 1),
                            )
                    for ci, (g, wsb, dst) in enumerate(chains):
                        nc.vector.tensor_copy(dst[:, ds(HALF * g, HALF)],
                                              pps[ci][:, :])
                    return qT, kT
                for g, wsb, dst in chains:
                    pp = psA.tile([P, HALF], F32, tag="ps",
                                  name=f"pp{pair}_{g}")
                    for c in range(CK):
                        nc.tensor.matmul(
                            pp[:, :],
                            wsb[:, c, :],
                            xt_sb[c][:, ds(HALF * g, HALF)],
                            start=(c == 0),
                            stop=(c == CK - 1),
                        )
                    nc.vector.tensor_copy(dst[:, ds(HALF * g, HALF)], pp[:, :])
                return qT, kT

            def scores(pair, qT, kT):
                """es tiles [128, s-tile, head, 512] per t-pass, exp'd + masked.

                Pass A (t < 512) emitted first so AV(hh=0) of this pair only
                waits ~3us of ScalarE exp, not the full ~11us."""
                esA = espA.tile([P, 4, 2, HALF], BF16, tag="esA", name=f"eA{pair}")
                esB = espB.tile([P, TT, 2, HALF], BF16, tag="esB", name=f"eB{pair}")
                for i in range(4):
                    # pass A: t in [t0, 512)  (only s-tiles in the first half)
                    t0 = P * i
                    wa = HALF - t0
                    sc = psSc.tile([P, 2, HALF], F32, tag="sc",
                                   name=f"scA{pair}_{i}")
                    for w in range(2):
                        nc.tensor.matmul(
                            sc[:, w, ds(t0, wa)],
                            kT[ds(HS * w, HS), ds(t0, P)],
                            qT[ds(HS * w, HS), ds(t0, wa)],
                        )
                    nc.scalar.activation(
                        esA[:, i, :, ds(t0, wa)],
                        sc[:, :, ds(t0, wa)],
                        mybir.ActivationFunctionType.Exp,
                        scale=HS ** -0.5,
                    )
                    nc.vector.tensor_tensor(
                        esA[:, i, :, ds(t0, P)], esA[:, i, :, ds(t0, P)],
                        maskrep[:, :, :], mybir.AluOpType.mult,
                    )
                for i in range(TT):
                    # pass B: t in [max(t0,512), 1024)
                    t0 = P * i
                    a0 = max(t0, HALF)
                    wb = T - a0
                    lb = a0 - HALF
                    sc = psSc.tile([P, 2, HALF], F32, tag="sc",
                                   name=f"scB{pair}_{i}")
                    for w in range(2):
                        nc.tensor.matmul(
                            sc[:, w, ds(lb, wb)],
                            kT[ds(HS * w, HS), ds(P * i, P)],
                            qT[ds(HS * w, HS), ds(a0, wb)],
                        )
                    nc.scalar.activation(
                        esB[:, i, :, ds(lb, wb)],
                        sc[:, :, ds(lb, wb)],
                        mybir.ActivationFunctionType.Exp,
                        scale=HS ** -0.5,
                    )
                    if i >= 4:
                        nc.vector.tensor_tensor(
                            esB[:, i, :, ds(lb, P)], esB[:, i, :, ds(lb, P)],
                            maskrep[:, :, :], mybir.AluOpType.mult,
                        )
                return esA, esB

            def attn_v(pair, esA, esB, late=False, tail=False):
                """out^T accumulation + normalize + DMA for both heads.

                DVE reciprocal cost scales with per-lane free size, so the
                [1, 1024] denominator row is DMA-repartitioned to [128, 8],
                recip'd there, and DMA'd back before the GpSimd broadcast."""
                for w in range(2):
                    h = 2 * pair + w
                    vt = v_all[pair // 4]
                    hg = h - 8 * (pair // 4)
                    avs = normp.tile([HS + 1, T], BF16, tag="avs",
                                     name=f"avs{h}")
                    for hh in range(2):
                        # the very last pair runs after exp(7) released the
                        # psSc banks; borrowing them doubles the av rotation
                        avp = psSc if (late and hh == w) else psV
                        av = avp.tile([HS + 1, HALF], F32,
                                      tag=("sc" if avp is psSc else "av"),
                                      name=f"av{h}_{hh}")
                        if hh == 0:
                            contrib = [(i, esA, P * i, HALF - P * i)
                                       for i in range(4)]
                        else:
                            contrib = [(i, esB, max(P * i, HALF) - HALF,
                                        T - max(P * i, HALF))
                                       for i in range(TT)]
                        for idx, (i, est, lo, wd) in enumerate(contrib):
                            nc.tensor.matmul(
                                av[:, ds(lo, wd)],
                                vt[:, hg, i, :],
                                est[:, i, w, ds(lo, wd)],
                                start=(idx == 0),
                                stop=(idx == len(contrib) - 1),
                            )
                        nc.vector.tensor_copy(
                            avs[:, ds(HALF * hh, HALF)], av[:, :])
                    _norm_chain(h, avs, 0, T, f"{h}", use_act=tail)

            def _norm_chain(h, avs, off, width, tag_sfx, use_act=False):
                rec = normp.tile([1, T], BF16, tag="rec", name=f"rec{tag_sfx}")
                if use_act:
                    # tail pairs: exp stream is over, the idle ScalarE does
                    # the [1, N] reciprocal directly -- no DMA round-trip
                    _act_recip(nc, rec[:, ds(off, width)],
                               avs[HS:HS + 1, ds(off, width)])
                else:
                    den_t = normp.tile([P, TT], BF16, tag="dent",
                                       name=f"dt{tag_sfx}")
                    nw = width // P
                    nc.sync.dma_start(den_t[:, 0:nw],
                                      avs[HS:HS + 1, ds(off, width)])
                    rec_t = normp.tile([P, TT], BF16, tag="rect",
                                       name=f"rt{tag_sfx}")
                    with nc.allow_low_precision(
                            reason="bf16 softmax denom: ~0.4% rel err, "
                            "within the 2e-2 gate"):
                        nc.vector.reciprocal(rec_t[:, 0:nw], den_t[:, 0:nw])
                    nc.sync.dma_start(rec[:, ds(off, width)], rec_t[:, 0:nw])
                rb = normp.tile([HS, T], BF16, tag="rb", name=f"rb{tag_sfx}")
                nc.gpsimd.partition_broadcast(
                    rb[:, ds(off, width)], rec[0:1, ds(off, width)])
                osb = normp.tile([HS, T], BF16, tag="osb", name=f"osb{tag_sfx}")
                nc.vector.tensor_tensor(
                    osb[:, ds(off, width)], avs[0:HS, ds(off, width)],
                    rb[:, ds(off, width)], mybir.AluOpType.mult,
                )
                nc.sync.dma_start(out[h, :, ds(off, width)],
                                  osb[:, ds(off, width)])

            # ---- v for one pair-group (8 heads), split by s-tile half ----
            def vproj(pg, jlo, jhi):
                for j in range(jlo, jhi):
                    pv = psA.tile([P, HALF], F32, tag="ps", name=f"pv{j}_{pg}")
                    for c in range(CK):
                        nc.tensor.matmul(
                            pv[:, :],
                            xt_sb[c][:, ts(j, P)],
                            wv_sb[:, c, pg, :],
                            start=(c == 0),
                            stop=(c == CK - 1),
                        )
                    # pv cols are (head0..head7 of group) x 64 in order
                    nc.vector.tensor_copy(
                        v_all[pg][:, :, j, 0:HS],
                        pv.rearrange("p (g d) -> p g d", d=HS),
                    )

            # ---- software-pipelined pair loop ----
            # v3-like interleave: AV(p-1) right after scores(p) so each
            # pair's normalize chain (DVE/DMA/GpSimd) hides under the next
            # projection.  The endgame keeps AV(5..7) (~11.4us of PE work)
            # after scores(7) to exactly cover exp(7) on ScalarE; only the
            # last head's normalize chain is exposed as tail.
            es = {}
            qk1 = proj(0, first=True)
            es[0] = scores(0, *qk1)
            qk1 = proj(1)
            es[1] = scores(1, *qk1)
            vproj(0, 0, TT)
            attn_v(0, *es.pop(0))
            w_sb[4] = dma_w(4)
            qk1 = proj(2)
            es[2] = scores(2, *qk1)
            attn_v(1, *es.pop(1))
            vproj(1, 0, 4)
            w_sb[5] = dma_w(5)
            qk1 = proj(3)
            es[3] = scores(3, *qk1)
            attn_v(2, *es.pop(2))
            vproj(1, 4, TT)
            w_sb[6] = dma_w(6)
            qk1 = proj(4)
            es[4] = scores(4, *qk1)
            attn_v(3, *es.pop(3))
            w_sb[7] = dma_w(7)
            qk1 = proj(5)
            es[5] = scores(5, *qk1)
            attn_v(4, *es.pop(4))
            qk1 = proj(6)
            es[6] = scores(6, *qk1)
            qk1 = proj(7)
            es[7] = scores(7, *qk1)
            attn_v(5, *es.pop(5), tail=True)
            attn_v(6, *es.pop(6), tail=True)
            attn_v(7, *es.pop(7), late=True, tail=True)
    nc.compile()
    return nc


def get_nc():
    global _BUILT
    if _BUILT is None:
        _BUILT = build_nc()
    return _BUILT


def prep_inputs(x, Wq, Wk, Wv):
    """Host-side shard + layout prep. Returns in_maps (one dict per core)."""
    x = np.asarray(x, dtype=np.float32)
    Wq = np.asarray(Wq, dtype=np.float32)
    Wk = np.asarray(Wk, dtype=np.float32)
    Wv = np.asarray(Wv, dtype=np.float32)
    bf = ml_dtypes.bfloat16

    # xT[b]: [C, T] -> [p, c, t] with row 128c+p
    xts = []
    for b in range(B):
        xT = np.ascontiguousarray(x[b].T)          # [C, T]
        xts.append(xT.reshape(CK, P, T).transpose(1, 0, 2).astype(bf))

    def pack_pairs(W):
        # [H, C, hs] -> [pair, C, 128] -> [pair, p, c, f]
        Wp = W.reshape(PAIRS, 2, C, HS).transpose(0, 2, 1, 3).reshape(PAIRS, C, P)
        return Wp.reshape(PAIRS, CK, P, P).transpose(0, 2, 1, 3)  # [pair, p, c, f]

    wq_p = pack_pairs(Wq)
    wk_p = pack_pairs(Wk)
    wqk_host = np.stack([wq_p, wk_p], axis=0).astype(bf)  # [2, pair, p, c, f]
    # wv: [p, c, pair, f]
    wv_host = np.ascontiguousarray(pack_pairs(Wv).transpose(1, 2, 0, 3)).astype(bf)

    return [
        {"xt": np.ascontiguousarray(xts[b]), "wqk": wqk_host, "wv": wv_host}
        for b in range(B)
    ]


def run_on_device(in_maps, **kwargs):
    nc = get_nc()
    return run_bass_kernel_spmd(nc, in_maps, list(range(B)), **kwargs)


def assemble(core_out):
    """[H, HS, T] out^T -> [T, H*HS]: pure layout transpose."""
    o = np.asarray(core_out, dtype=np.float32)
    return np.ascontiguousarray(o.transpose(2, 0, 1).reshape(T, H * HS))


def kernel(x, Wq, Wk, Wv):
    in_maps = prep_inputs(x, Wq, Wk, Wv)
    res = run_on_device(in_maps)
    return np.stack([assemble(res.results[b]["out"]) for b in range(B)], axis=0)
